# revision 8
# baseline (speedup 1.0000x reference)
"""Trainium2 Bass kernel for nn_BaselineTargetHead (per-sample dynamic MLP).

Strategy: data-parallel over 8 NeuronCores, 8 samples per core.
Per sample the chain is 5 per-sample linear layers over 64 spatial positions:
  [1024,2048] @ [2048,64] -> sigmoid -> ... -> [1,128] @ [128,64] + b

The kernel is HBM-bandwidth bound on per-sample weight traffic, so fc1-fc4
weights (99.9% of bytes) and the input x ship as fp8 e3m4 (4 mantissa bits).
Host pre-scales weights by 64 (x by 2) to center N(0,0.02) data in e3m4's
normal range; the inverse scale folds into the ScalarE activation's `scale`.
fc5 weights stay fp16: the output is a 128-term dot product with no
downstream averaging, so fc5 quantization dominates the error budget
(measured: quantizing w5 alone costs 1.4e-2 rel err; w1-w4 cost ~1e-3).

Device kernel (per core, per sample):
  - weights arrive as pre-transposed "slabs" laid out exactly as the SBUF
    image [128 part, sum_l (Cin_l/128)*Cout_l cols] so one large contiguous
    DMA per sample loads its full weight set (double-buffered). Sample 0's
    fc1 is split into four chunks so the PE starts ~5us earlier.
  - matmul: lhsT = W^T tile [128(Cin), 128(Cout)] fp8, rhs = activation tile
    [128(Cin), 64(spatial)] fp16, accumulate over Cin tiles in PSUM fp32.
    fp8 stationary weights also halve LDWEIGHTS time via FWL.
  - ScalarE applies scale+bias+sigmoid fused, writing fp16 activation tiles
    that feed the next layer without any transposition. All non-weight DMAs
    ride the Vector engine's queue so ScalarE only runs activations.
"""

import numpy as np
import ml_dtypes

import concourse.bass as bass
import concourse.mybir as mybir
import concourse.tile as tile
from concourse.bass_utils import run_bass_kernel_spmd

N_CORES = 8
B = 64
S_PER_CORE = B // N_CORES  # 8 samples per core
HW = 64  # 8x8 spatial positions
LAYERS = [(2048, 1024), (1024, 512), (512, 256), (256, 128)]  # (Cin, Cout) of fc1..fc4
W_SCALE_FP8 = 64.0  # host multiplies fp8 weights by this; kernel divides back
X_SCALE_FP8 = 2.0  # same for the input x image
W8_COLS = sum((ci // 128) * co for ci, co in LAYERS)  # 21760
A_COLS = (LAYERS[0][0] // 128) * LAYERS[0][1]  # 16384 (fc1)
X_COLS = (2048 // 128) * HW  # 1024
W5_COLS = 32  # w5 zero-padded to 32 cols for a legal M=32 matmul
# bias image columns per sample: fc1 m0..7 | fc2 m0..3 | fc3 m0..1 | fc4 m0 | fc5
BIAS_COL0 = [0, 8, 12, 14]
BIAS_COLS = 16
# per-layer PSUM scale to undo the host-side fp8 pre-scaling
ACT_SCALE = [
    1.0 / (W_SCALE_FP8 * X_SCALE_FP8),
    1.0 / W_SCALE_FP8,
    1.0 / W_SCALE_FP8,
    1.0 / W_SCALE_FP8,
]


def _split_ctrl_multiwaits(nc):
    """walrus in this env rejects >1 sync-wait per instruction. Move extra
    waits onto NOPs placed immediately before, on the same engine — engines
    execute in order, so this is semantically identical."""
    n_fixed = 0
    for bb in nc.main_func.blocks:
        insts = bb.instructions
        i = 0
        while i < len(insts):
            ins = insts[i]
            si = ins.sync_info
            if si is not None and si.on_wait and len(si.on_wait) > 1:
                waits = list(si.on_wait)
                new_nops = []
                for j, w in enumerate(waits[1:]):
                    nop = mybir.InstNoOp(name=f"{ins.name}-splitw-{j}", ins=[], outs=[])
                    nop.engine = ins.engine
                    nop.sync_info = mybir.SyncInfo(on_update=[], on_wait=[w])
                    new_nops.append(nop)
                si.on_wait = [waits[0]]
                insts[i:i] = new_nops
                i += len(new_nops)
                n_fixed += 1
            i += 1
    return n_fixed


def _build_nc():
    f8 = mybir.dt.float8e3
    f16 = mybir.dt.float16
    f32 = mybir.dt.float32
    nc = bass.Bass()
    wslab_d = nc.dram_tensor("wslab", [S_PER_CORE, 128, W8_COLS], f8, kind="ExternalInput")
    ximg_d = nc.dram_tensor("ximg", [128, S_PER_CORE * X_COLS], f8, kind="ExternalInput")
    w5img_d = nc.dram_tensor("w5img", [128, S_PER_CORE * W5_COLS], f16, kind="ExternalInput")
    bias_d = nc.dram_tensor("bias", [128, S_PER_CORE * BIAS_COLS], f32, kind="ExternalInput")
    out_d = nc.dram_tensor("out", [1, S_PER_CORE * HW], f32, kind="ExternalOutput")

    sig = mybir.ActivationFunctionType.Sigmoid
    ident = mybir.ActivationFunctionType.Identity

    with tile.TileContext(nc) as tc:
        with (
            tc.tile_pool(name="wpool", bufs=3) as wpool,
            tc.tile_pool(name="qpool", bufs=2) as qpool,
            tc.tile_pool(name="misc", bufs=1) as misc,
            tc.tile_pool(name="psum", bufs=6, space="PSUM") as psum_pool,
        ):
            # non-weight inputs ride the Vector engine's HWDGE queue so the
            # SP queue carries nothing but the weight-slab stream and ScalarE
            # only runs activations. x first: it gates sample 0's fc1.
            x_sb = misc.tile([128, S_PER_CORE * X_COLS], f8)
            nc.scalar.dma_start(x_sb[:], ximg_d[:])
            bias_sb = misc.tile([128, S_PER_CORE * BIAS_COLS], f32)
            nc.scalar.dma_start(bias_sb[:], bias_d[:])
            w5_sb = misc.tile([128, S_PER_CORE * W5_COLS], f16)
            nc.scalar.dma_start(w5_sb[:], w5img_d[:])
            # all samples' outputs land in partition 0 of one tile
            # (sample s -> columns s*HW..(s+1)*HW) so one DMA ships them all
            ot_all = misc.tile([128, S_PER_CORE * HW], f32)

            # per-layer column offset within the fc2-4 slab tile
            layer_off = [0, 0]
            for cin, cout in LAYERS[1:-1]:
                layer_off.append(layer_off[-1] + (cin // 128) * cout)

            for s in range(S_PER_CORE):
                if s == 0:
                    # fine-grained so the PE starts ~5us earlier on sample 0
                    QA = A_COLS // 4
                    wta_parts = []
                    for j in range(4):
                        t = wpool.tile([128, QA], f8, tag=f"wslabA{j}")
                        nc.sync.dma_start(t[:], wslab_d[s, :, j * QA : (j + 1) * QA])
                        wta_parts.append(t)

                    def wa(col, parts=wta_parts):
                        return parts[col // QA], col % QA
                else:
                    t = wpool.tile([128, A_COLS], f8, tag="wslabA")
                    nc.sync.dma_start(t[:], wslab_d[s, :, 0:A_COLS])

                    def wa(col, t=t):
                        return t, col
                wtb = wpool.tile([128, W8_COLS - A_COLS], f8, tag="wslabB")
                nc.sync.dma_start(wtb[:], wslab_d[s, :, A_COLS:W8_COLS])

                q_prev = x_sb[:, s * X_COLS : (s + 1) * X_COLS]
                for li, (cin, cout) in enumerate(LAYERS):
                    kt, mt = cin // 128, cout // 128
                    off = layer_off[li]
                    qn = qpool.tile([128, mt * HW], f16, tag=f"q{li}")
                    for m in range(mt):
                        ps = psum_pool.tile([128, HW], f32, tag="ps")
                        for k in range(kt):
                            if li == 0:
                                wt, wcol = wa(k * cout + m * 128)
                            else:
                                wt, wcol = wtb, off + k * cout + m * 128
                            lhsT = wt[:, wcol : wcol + 128]
                            rhs = q_prev[:, k * HW : (k + 1) * HW]
                            nc.tensor.matmul(
                                ps[:], lhsT, rhs, start=(k == 0), stop=(k == kt - 1)
                            )
                        bcol = s * BIAS_COLS + BIAS_COL0[li] + m
                        nc.scalar.activation(
                            qn[:, m * HW : (m + 1) * HW],
                            ps[:],
                            sig,
                            bias=bias_sb[:, bcol : bcol + 1],
                            scale=ACT_SCALE[li],
                        )
                    q_prev = qn[:]

                ps5 = psum_pool.tile([128, HW], f32, tag="ps", name=f"ps5_{s}")
                w5t = w5_sb[:, s * W5_COLS : (s + 1) * W5_COLS]
                nc.tensor.matmul(
                    ps5[0:32, :], w5t, q_prev[:, 0:HW], start=True, stop=True
                )
                b5col = s * BIAS_COLS + 15
                nc.scalar.activation(
                    ot_all[0:1, s * HW : (s + 1) * HW],
                    ps5[0:1, :],
                    ident,
                    bias=bias_sb[0:1, b5col : b5col + 1],
                    scale=1.0,
                )
            nc.scalar.dma_start(out_d[:, :], ot_all[0:1, :])

    _split_ctrl_multiwaits(nc)
    return nc


_NC_CACHE = None


def _get_nc():
    global _NC_CACHE
    if _NC_CACHE is None:
        _NC_CACHE = _build_nc()
    return _NC_CACHE


def _to_e3m4(a, scale):
    return np.clip(a * scale, -14.0, 14.0).astype(ml_dtypes.float8_e3m4)


def _prep_core(inputs, c):
    """Build the per-core input map (numpy only, host-side layout prep)."""
    sl = slice(c * S_PER_CORE, (c + 1) * S_PER_CORE)

    def wimg(li):
        cin, cout = LAYERS[li]
        w = inputs[f"target_fc{li + 1}w"][sl, :, :, 0, 0]  # [S, Cout, Cin]
        # -> [S, 128, (Cin/128)*Cout] with img[s, p, k*Cout+co] = w[s, co, k*128+p]
        wt = w.transpose(0, 2, 1).reshape(S_PER_CORE, cin // 128, 128, cout)
        return wt.transpose(0, 2, 1, 3).reshape(S_PER_CORE, 128, -1)

    wslab = np.ascontiguousarray(
        _to_e3m4(np.concatenate([wimg(li) for li in range(len(LAYERS))], axis=2), W_SCALE_FP8)
    )

    x = inputs["target_in_vec"][sl].reshape(S_PER_CORE, 2048 // 128, 128, HW)
    ximg = x.transpose(2, 0, 1, 3).reshape(128, S_PER_CORE * X_COLS)
    ximg = np.ascontiguousarray(_to_e3m4(ximg, X_SCALE_FP8))

    w5 = inputs["target_fc5w"][sl, 0, :, 0, 0].astype(np.float16)  # [S, 128]
    w5img = np.zeros((128, S_PER_CORE, W5_COLS), np.float16)
    w5img[:, :, 0] = w5.T
    w5img = np.ascontiguousarray(w5img.reshape(128, -1))

    bias = np.zeros((S_PER_CORE, 128, BIAS_COLS), np.float32)
    for li, (cin, cout) in enumerate(LAYERS):
        b = inputs[f"target_fc{li + 1}b"][sl]  # [S, Cout]
        bias[:, :, BIAS_COL0[li] : BIAS_COL0[li] + cout // 128] = b.reshape(
            S_PER_CORE, cout // 128, 128
        ).transpose(0, 2, 1)
    bias[:, 0, 15] = inputs["target_fc5b"][sl, 0]
    bias = np.ascontiguousarray(bias.transpose(1, 0, 2).reshape(128, -1))

    return {"wslab": wslab, "ximg": ximg, "w5img": w5img, "bias": bias}


def kernel(**inputs):
    inputs = {k: np.asarray(v) for k, v in inputs.items()}
    nc = _get_nc()
    in_maps = [_prep_core(inputs, c) for c in range(N_CORES)]
    res = run_bass_kernel_spmd(nc, in_maps, list(range(N_CORES)))
    out = np.concatenate([np.asarray(res.results[c]["out"]) for c in range(N_CORES)], axis=0)
    return out.reshape(B, 8, 8).astype(np.float32)
